# revision 3
# baseline (speedup 1.0000x reference)
"""Trainium2 Bass kernel for out = exp(-M) @ x.

M: [16384, 16384] fp32, x: [16384, 128] fp32 -> out: [16384, 128] fp32.

Sharding: row-shard M and out over 8 cores (2048 rows each), x replicated.

Per-core pipeline (all engines overlapped, DMA-bound at ~128 MiB HBM reads):
  DMA   : M tiles [128, 2048] fp32, natural layout (8 KiB contiguous rows)
  ACT   : e = exp(-M_tile) fused fp32 -> bf16 (free affine scale=-1)
  PE    : transpose e chunks [128m, 128k] -> PSUM [128k, 128m] (bf16)
  DVE   : evacuate PSUM -> SBUF rhs tiles [128k, 512m]
  PE    : out.T[f, m] += x[kchunk].T @ rhs   (x stationary bf16, fp32 PSUM acc)
  PE/DVE: final [f, m] -> [m, f] transpose, DMA store
"""

import sys

sys.path.insert(0, "/opt/trn_rl_repo")

import numpy as np

import concourse.bass as bass  # noqa: F401  (engine namespaces live on nc)
import concourse.mybir as mybir
import concourse.tile as tile
from concourse import bacc
from concourse.bass_utils import run_bass_kernel_spmd
from concourse.masks import make_identity

N = 16384  # M is [N, N]
D = 128  # x is [N, D]
N_CORES = 8
M_ROWS = N // N_CORES  # 2048 rows of M / out per core

F32 = mybir.dt.float32
BF16 = mybir.dt.bfloat16
EXP = mybir.ActivationFunctionType.Exp

# geometry
M_SUPER = 512  # output rows accumulated per PSUM bank
N_SUPERS = M_ROWS // M_SUPER  # 4
K_WIN = 2048  # contraction window per M DMA tile
N_WINS = N // K_WIN  # 8
M_SUBS = M_SUPER // 128  # 4 m-subtiles per super
KC_PER_WIN = K_WIN // 128  # 16 k-chunks per window
N_KCHUNKS = N // 128  # 128 total k-chunks


def build_kernel():
    nc = bacc.Bacc("TRN2", target_bir_lowering=False, debug=False)
    m_ap = nc.dram_tensor("m_shard", [M_ROWS, N], F32, kind="ExternalInput").ap()
    x_ap = nc.dram_tensor("x", [N, D], F32, kind="ExternalInput").ap()
    out_ap = nc.dram_tensor("out", [M_ROWS, D], F32, kind="ExternalOutput").ap()

    from contextlib import ExitStack

    with tile.TileContext(nc) as tc, ExitStack() as ctx:
        consts = ctx.enter_context(tc.tile_pool(name="consts", bufs=1))
        ident_bf = consts.tile([128, 128], BF16)
        make_identity(nc, ident_bf[:])
        ident_f32 = consts.tile([128, 128], F32)
        make_identity(nc, ident_f32[:])

        # x resident in SBUF as bf16, chunk c at xbf[:, c*128:(c+1)*128]
        # (partition = k within chunk, free = feature)
        xbf_t = consts.tile([128, N_KCHUNKS * D], BF16)
        with tc.tile_pool(name="xstage", bufs=4) as xstage:
            for c in range(N_KCHUNKS):
                xs = xstage.tile([128, D], F32)
                nc.sync.dma_start(out=xs[:], in_=x_ap[c * 128 : (c + 1) * 128, :])
                nc.vector.tensor_copy(xbf_t[:, c * D : (c + 1) * D], xs[:])

        m_pool = ctx.enter_context(tc.tile_pool(name="m", bufs=6))
        e_pool = ctx.enter_context(tc.tile_pool(name="e", bufs=8))
        rhs_pool = ctx.enter_context(tc.tile_pool(name="rhs", bufs=4))
        outT_pool = ctx.enter_context(tc.tile_pool(name="outT", bufs=2))
        outf_pool = ctx.enter_context(tc.tile_pool(name="outf", bufs=2))
        pt_pool = ctx.enter_context(tc.tile_pool(name="pt", bufs=4, space="PSUM"))
        pout_pool = ctx.enter_context(tc.tile_pool(name="pout", bufs=2, space="PSUM"))
        pfin_pool = ctx.enter_context(tc.tile_pool(name="pfin", bufs=2, space="PSUM"))

        for ms in range(N_SUPERS):
            pout = pout_pool.tile([128, M_SUPER], F32)
            for kw in range(N_WINS):
                ebf = []
                for j in range(M_SUBS):
                    mt = m_pool.tile([128, K_WIN], F32)
                    r0 = ms * M_SUPER + j * 128
                    nc.sync.dma_start(
                        out=mt[:],
                        in_=m_ap[r0 : r0 + 128, kw * K_WIN : (kw + 1) * K_WIN],
                    )
                    e = e_pool.tile([128, K_WIN], BF16)
                    nc.scalar.activation(e[:], mt[:], EXP, scale=-1.0)
                    ebf.append(e)
                for kc in range(KC_PER_WIN):
                    kg = kw * KC_PER_WIN + kc
                    pt = pt_pool.tile([128, M_SUPER], BF16)
                    for j in range(M_SUBS):
                        nc.tensor.transpose(
                            pt[:, j * 128 : (j + 1) * 128],
                            ebf[j][:, kc * 128 : (kc + 1) * 128],
                            ident_bf[:],
                        )
                    rhs = rhs_pool.tile([128, M_SUPER], BF16)
                    nc.vector.tensor_copy(rhs[:], pt[:])
                    nc.tensor.matmul(
                        pout[:],
                        lhsT=xbf_t[:, kg * D : (kg + 1) * D],
                        rhs=rhs[:],
                        start=(kg == 0),
                        stop=(kg == N_KCHUNKS - 1),
                    )
            # evacuate out.T [f, m] and transpose to [m, f]
            outT = outT_pool.tile([128, M_SUPER], F32)
            nc.vector.tensor_copy(outT[:], pout[:])
            for j in range(M_SUBS):
                pf = pfin_pool.tile([128, D], F32)
                nc.tensor.transpose(
                    pf[:], outT[:, j * 128 : (j + 1) * 128], ident_f32[:]
                )
                of = outf_pool.tile([128, D], F32)
                nc.vector.tensor_copy(of[:], pf[:])
                r0 = ms * M_SUPER + j * 128
                nc.sync.dma_start(out=out_ap[r0 : r0 + 128, :], in_=of[:])

    nc.compile()
    return nc


_NC_CACHE = None


def _get_nc():
    global _NC_CACHE
    if _NC_CACHE is None:
        _NC_CACHE = build_kernel()
    return _NC_CACHE


def kernel(M, x):
    M = np.ascontiguousarray(np.asarray(M, dtype=np.float32))
    x = np.ascontiguousarray(np.asarray(x, dtype=np.float32))
    assert M.shape == (N, N) and x.shape == (N, D)
    nc = _get_nc()
    in_maps = [
        {"m_shard": M[c * M_ROWS : (c + 1) * M_ROWS], "x": x} for c in range(N_CORES)
    ]
    res = run_bass_kernel_spmd(nc, in_maps, list(range(N_CORES)))
    return np.concatenate([res.results[c]["out"] for c in range(N_CORES)], axis=0)
